# revision 1
# baseline (speedup 1.0000x reference)
"""MoE (top-2 of 8 experts, d=1024, h=4096) on 8 Trainium2 NeuronCores.

Strategy (expert-parallel, per sharding hint):
  - Host: gating (fp64 logits/softmax/top-2 — tie margins on this problem are
    ~1e-5, far above fp32 rounding noise, so host routing matches the
    reference's fp32 top-k), per-expert token gather, pad to capacity C.
  - Device (core e = expert e): hidT = relu(W1_e.T @ x_e.T + b1_e) then
    ye = hidT.T @ W2_e, both as K-tiled 128x128x512 matmuls in float32r
    (full PE rate, ~1e-4 matmul rel err).
  - Host: out[tok_e] += gate_e * (ye + b2_e)  (scatter-combine).

Self-contained: hardcodes all shapes; only imports concourse (system lib).
"""

import os

os.environ.setdefault("JAX_PLATFORMS", "")

import numpy as np

import concourse.bacc as bacc
import concourse.mybir as mybir
import concourse.tile as tile
from concourse.bass_utils import run_bass_kernel_spmd

P = 128
D = 1024  # embed dim
H = 4096  # hidden dim
E = 8  # experts
TOPK = 2
KD = D // P  # 8  k-tiles over embed
KH = H // P  # 32 k-tiles over hidden
NCORES = 8
FD = 512  # matmul moving free dim (one PSUM bank of fp32)

_compiled = {}
LAST_RESULT = None  # BassKernelResults of the most recent run (for test harness)


def _phase1(nc, tc, rs, C, chunks, xt_d, w1_d, b1_d, hid_cs):
    f32 = mybir.dt.float32
    f32r = mybir.dt.float32r
    relu = mybir.ActivationFunctionType.Relu
    TN = len(chunks)
    with (
        tc.tile_pool(name=rs + "xts_p", bufs=1) as xts_p,
        tc.tile_pool(name=rs + "b1_p", bufs=1) as b1_p,
        tc.tile_pool(name=rs + "w1_p", bufs=int(os.environ.get("MOE_W1B", "4"))) as w1_p,
        tc.tile_pool(name=rs + "hb_p", bufs=1) as hb_p,
        tc.tile_pool(name=rs + "ps1", bufs=int(os.environ.get("MOE_PS1", "4")), space="PSUM") as ps1,
    ):
        def load_w1(hm):
            w1t = w1_p.tile([P, KD, P], f32r, tag="w1t", name=rs + f"w1t_{hm}")
            nc.sync.dma_start(w1t[:], w1_d[:, hm])
            return w1t

        # Issue order matters: HWDGE dispatches in program order, so the
        # first matmul group's deps (w1t[0] + x chunk 0) are issued first.
        w1_pre = {0: load_w1(0)}
        # x chunks as separate per-k tiles so the first matmul group only
        # waits on its own 8 pieces (~2MB), not the whole 10MB load.
        xc = [[None] * KD for _ in range(TN)]
        for tn, (off, w) in enumerate(chunks):
            for k0 in range(0, KD, 2):
                t = xts_p.tile(
                    [P, 2, w], f32r, tag=f"x_{tn}_{k0}", name=rs + f"x_{tn}_{k0}"
                )
                nc.sync.dma_start(t[:], xt_d[:, k0 : k0 + 2, off : off + w])
                xc[tn][k0] = t[:, 0, :]
                xc[tn][k0 + 1] = t[:, 1, :]
            if tn == 0:
                # b1 is first needed at the first eviction, not the first
                # matmul: issue it after chunk 0's loads.
                b1s = b1_p.tile([P, KH], f32, name=rs + "b1s")
                nc.sync.dma_start(b1s[:], b1_d[:])
            if tn < 3:  # prefetch next stationary tiles early
                w1_pre[tn + 1] = load_w1(tn + 1)
        # PE emission order: the first W hm rows are swept tn-major (wave
        # order) so the earliest matmuls only touch x chunks that have
        # already landed; the rest are hm-major. Each (hm, tn) psum group is
        # independent, so this only reorders work.
        W = int(os.environ.get("MOE_W", "2")) if TN > 1 else 0
        sched = [(hm, tn) for tn in range(TN) for hm in range(W)]
        sched += [(hm, tn) for hm in range(W, KH) for tn in range(TN)]

        w1ts, done = {}, {}
        KQ1 = KH // 4
        for hm, tn in sched:
            if hm not in w1ts:
                w1ts[hm] = w1_pre.pop(hm) if hm in w1_pre else load_w1(hm)
                done[hm] = 0
            off, w = chunks[tn]
            pt = ps1.tile([P, FD], f32, tag="ps1", name=rs + f"ps1_{hm}_{tn}")
            for k in range(KD):
                nc.tensor.matmul(
                    pt[:, :w],
                    w1ts[hm][:, k, :],
                    xc[tn][k],
                    start=(k == 0),
                    stop=(k == KD - 1),
                )
            # evict through a small per-chunk staging tile (ACT does
            # relu+bias, then the hid write DMAs it straight out on the ACT
            # HWDGE ring so phase-2 loads (SP ring) aren't queued behind it)
            hbst = int(os.environ.get("MOE_HBST", "12")) if C <= 2560 else 6
            hb = hb_p.tile([P, w], f32r, tag="hbst", bufs=hbst, name=rs + f"hb_{hm}_{tn}")
            nc.scalar.activation(
                hb[:, :w], pt[:, :w], relu, bias=b1s[:, hm : hm + 1]
            )
            nc.scalar.dma_start(
                hid_cs[tn][hm // KQ1][:, :, hm % KQ1, :].transpose([1, 0, 2]),
                hb.rearrange("p (t q) -> p t q", q=P),
            )
            done[hm] += 1
            if done[hm] == TN:
                del w1ts[hm]  # release references; pool slots recycle


W2HEAD = 8  # w2 chunks living in the persistent pool (loadable during phase 1)


def _phase2(nc, tc, rs, C, chunks, w2_d, hid_cs, ye_d, hd_p, w2h_p, ps2):
    f32 = mybir.dt.float32
    f32r = mybir.dt.float32r
    TM = C // P
    with (
        tc.tile_pool(name=rs + "w2_p", bufs=1) as w2_p,
        tc.tile_pool(name=rs + "out_p", bufs=int(os.environ.get("MOE_OUTB", "3"))) as out_p,
    ):

        HDS = 4  # hd k-split (must match the 4-way hid_cs DRAM split)
        KQ = KH // HDS

        def load_hd(tm):
            cidx = next(
                i for i, (off, w) in enumerate(chunks) if off // P <= tm < (off + w) // P
            )
            local = tm - chunks[cidx][0] // P
            parts = []
            for q in range(HDS):
                hdq = hd_p.tile(
                    [P, KQ, P], f32r, tag=f"hd{q}", name=rs + f"hd_{tm}_{q}"
                )
                nc.sync.dma_start(hdq[:], hid_cs[cidx][q][local])
                parts.append(hdq)
            return parts

        # Issue order: w2 head + first token tile's data before the bulk w2
        # load, so the first phase-2 matmul isn't queued behind 16MB of w2 on
        # the in-order HWDGE ring. Head w2 + hd live in pools hoisted outside
        # phase 1's, so these loads can run during phase 1's tail.
        w2ts = []
        for k in range(W2HEAD):
            w2t = w2h_p.tile([P, D], f32r, tag=f"w2_{k}", name=rs + f"w2_{k}")
            nc.sync.dma_start(w2t[:], w2_d[k])
            w2ts.append(w2t)
        hd_pre = {0: load_hd(0)}
        for k in range(W2HEAD, KH):
            w2t = w2_p.tile([P, D], f32r, tag=f"w2_{k}", name=rs + f"w2_{k}")
            nc.sync.dma_start(w2t[:], w2_d[k])
            w2ts.append(w2t)
            if k == 15:
                hd_pre[1] = load_hd(1)
        hd_pre[2] = load_hd(2)
        for tm in range(TM):
            hd = hd_pre.pop(tm) if tm in hd_pre else load_hd(tm)
            ob = out_p.tile([P, D], f32, tag="ob", name=rs + f"ob_{tm}")
            for n in range(D // FD):
                pt2 = ps2.tile([P, FD], f32, tag="ps2", name=rs + f"ps2_{tm}_{n}")
                for k in range(KH):
                    nc.tensor.matmul(
                        pt2[:],
                        hd[k // KQ][:, k % KQ, :],
                        w2ts[k][:, n * FD : (n + 1) * FD],
                        start=(k == 0),
                        stop=(k == KH - 1),
                    )
                nc.vector.tensor_copy(ob[:, n * FD : (n + 1) * FD], pt2[:])
            nc.scalar.dma_start(ye_d[tm], ob[:])


def _build(C, reps=1):
    """Per-core SPMD program for capacity-C tokens through one expert.

    reps>1 repeats the whole program back-to-back (timing experiments only).
    """
    if (C, reps) in _compiled:
        return _compiled[(C, reps)]
    f32 = mybir.dt.float32
    f32r = mybir.dt.float32r
    TM = C // P  # token tiles (GEMM2 stationary / output rows)
    # GEMM1 moving chunks: 512s plus one remainder (multiple of 128; N>=256
    # keeps fp32r at full rate, a 128 tail is negligible)
    chunks = []
    off = 0
    CW = int(os.environ.get("MOE_CW", "0"))
    if CW and C % CW == 0:  # uniform chunk-width experiment knob
        while off < C:
            chunks.append((off, CW))
            off += CW
    else:
        if C >= 768:  # small first chunk -> first matmul group starts sooner
            chunks.append((0, 256))
            off = 256
        while off < C:
            w = min(FD, C - off)
            chunks.append((off, w))
            off += w

    nc = bacc.Bacc(None, target_bir_lowering=False)
    # xt host layout [P, KD, C]: xt[p, k, c] = x[tok_c, k*128+p] (transposed)
    xt_d = nc.dram_tensor("xt", [P, KD, C], f32r, kind="ExternalInput")
    # w1 host layout [P, KH, KD, P]: w1[p, hm, k, j] = W1[k*128+p, hm*128+j]
    # -> per-hm stationary-tile loads are contiguous 4KB per partition.
    w1_d = nc.dram_tensor("w1", [P, KH, KD, P], f32r, kind="ExternalInput")
    b1_d = nc.dram_tensor("b1", [P, KH], f32, kind="ExternalInput")
    w2_d = nc.dram_tensor("w2", [KH, P, D], f32r, kind="ExternalInput")
    ye_d = nc.dram_tensor("ye", [TM, P, D], f32, kind="ExternalOutput")

    with tile.TileContext(nc) as tc:
        with tc.tile_pool(name="dram", bufs=1, space="DRAM") as dram:
            # hidT blocks: [token-tile, hidden-in-tile (partition), hm,
            # token-in-tile] -> phase-2 reads are contiguous 16KB/partition.
            # One DRAM tile per token chunk so phase-2's first loads only
            # depend on writes to their own chunk.
            # ... and per k-quarter, so phase-2's early hd quarters depend
            # only on the phase-1 rows that produced them (DRAM deps are
            # whole-tile).
            hid_cs = [
                [
                    dram.tile(
                        [w // P, P, KH // 4, P],
                        f32r,
                        tag=f"hidc_{i}_{q}",
                        name=f"hidc_{i}_{q}",
                    )
                    for q in range(4)
                ]
                for i, (off, w) in enumerate(chunks)
            ]
            for rep in range(reps):
                rs = "" if rep == 0 else f"r{rep}_"
                # hd/w2-head/psum2 pools are hoisted outside phase 1's pools
                # so phase 2's first loads don't wait for phase-1 SBUF release.
                with (
                    tc.tile_pool(name=rs + "hd_p", bufs=3) as hd_p,
                    tc.tile_pool(name=rs + "w2h_p", bufs=1) as w2h_p,
                    tc.tile_pool(name=rs + "ps2", bufs=int(os.environ.get("MOE_PS2", "4")), space="PSUM") as ps2,
                ):
                    _phase1(nc, tc, rs, C, chunks, xt_d, w1_d, b1_d, hid_cs)
                    _phase2(
                        nc, tc, rs, C, chunks, w2_d, hid_cs, ye_d, hd_p, w2h_p, ps2
                    )

    nc.compile()
    _compiled[(C, reps)] = nc
    return nc


def kernel(x, Wg, bg, W1, b1, W2, b2):
    global LAST_RESULT
    x = np.ascontiguousarray(x, dtype=np.float32)
    B, S, d = x.shape
    assert d == D
    T = B * S
    xf = x.reshape(T, d)

    # ---- Host gating/routing (fp64) ----
    logits = xf.astype(np.float64) @ Wg.astype(np.float64) + bg.astype(np.float64)
    mx = logits.max(axis=1, keepdims=True)
    ex = np.exp(logits - mx)
    probs = ex / ex.sum(axis=1, keepdims=True)
    order = np.argsort(-logits, axis=1, kind="stable")  # ties -> lower index
    top = order[:, :TOPK]  # [T, 2]
    gsel = np.take_along_axis(probs, top, axis=1).astype(np.float32)

    toks, gates = [], []
    for e in range(E):
        pos = top == e  # [T, 2]
        sel = pos.any(axis=1)
        toks.append(np.nonzero(sel)[0])
        gates.append((gsel * pos).sum(axis=1)[sel].astype(np.float32))

    maxcnt = max(len(t) for t in toks)
    # SBUF budget caps resident x at 4096 tokens/core; batch if routing is
    # ever concentrated enough to exceed that (never for balanced gating).
    MAXC = 2944
    nb = max(1, -(-maxcnt // MAXC))
    C = max(P, ((-(-maxcnt // nb) + P - 1) // P) * P)

    w_maps = []  # per-expert weight shards (batch-invariant)
    for e in range(E):
        w_maps.append(
            {
                "w1": np.ascontiguousarray(
                    np.asarray(W1[e], dtype=np.float32)
                    .reshape(KD, P, KH, P)
                    .transpose(1, 2, 0, 3)
                ),
                "b1": np.ascontiguousarray(
                    np.asarray(b1[e], dtype=np.float32).reshape(KH, P).T
                ),
                "w2": np.ascontiguousarray(W2[e], dtype=np.float32).reshape(KH, P, D),
            }
        )

    nc = _build(C)
    out = np.zeros((T, D), np.float32)
    b2f = np.asarray(b2, dtype=np.float32)
    for b in range(nb):
        in_maps = []
        btoks = []
        for e in range(E):
            tk = toks[e][b * C : (b + 1) * C]
            btoks.append(tk)
            xe = np.zeros((C, D), np.float32)
            xe[: len(tk)] = xf[tk]
            in_maps.append(
                {
                    "xt": np.ascontiguousarray(
                        xe.T.reshape(KD, P, C).transpose(1, 0, 2)
                    ),
                    **w_maps[e],
                }
            )
        res = run_bass_kernel_spmd(nc, in_maps, core_ids=list(range(NCORES)))
        LAST_RESULT = res
        for e in range(E):
            cnt = len(btoks[e])
            if cnt == 0:
                continue
            ye = res.results[e]["ye"].reshape(C, D)[:cnt]
            g = gates[e][b * C : b * C + cnt]
            out[btoks[e]] += g[:, None] * (ye + b2f[e])
    return out.reshape(B, S, D)



# revision 18
# speedup vs baseline: 1.1596x; 1.1596x over previous
"""MoE (top-2 of 8 experts, d=1024, h=4096) on 8 Trainium2 NeuronCores.

Strategy (expert-parallel with load-balanced spill slots):
  - Host: gating (fp64 logits/softmax/top-2; tie margins ~1e-5 >> fp32 noise).
  - Each core runs the same program with three single-expert "slots" of fixed
    token capacity (1024, 960, 128) = C 2112.  Each expert fills its own
    core's slots first; the three oversized experts spill their overflow into
    128-token slots borrowed from underloaded cores.  This caps the per-core
    capacity near the mean (2112 vs 2304 for naive expert-per-core).
  - Device per slot (fused, hid stays in SBUF): G1: hidT = relu(W1_s.T @ x_s
    + b1_s) per 128-row tile straight out of PSUM; G2: ye = hidT.T @ W2_s with
    W2 streamed.  All matmuls bf16 (slot 2's G2 in fp8e4 so its weights can be
    fully prefetched; only ~450 of 16384 token-assignments touch fp8).
  - Host: out[tok] += gate * (ye + b2)  (scatter-combine, fp32).

Self-contained: hardcodes all shapes; only imports concourse (system lib).
"""

import os

os.environ.setdefault("JAX_PLATFORMS", "")

import numpy as np
import ml_dtypes

import concourse.bacc as bacc
import concourse.mybir as mybir
import concourse.tile as tile
from concourse.bass_utils import run_bass_kernel_spmd

P = 128
D = 1024  # embed dim
H = 4096  # hidden dim
E = 8  # experts
TOPK = 2
KD = D // P  # 8  k-tiles over embed
KH = H // P  # 32 k-tiles over hidden
NCORES = 8

BF16 = ml_dtypes.bfloat16
FP8 = ml_dtypes.float8_e4m3
W2Q_SCALE = 64.0  # host pre-scale so fp8 W2 values sit in the normal range

# Per-core slot capacities.  Slot 2 (the spill slot) runs its G2 in fp8.
CAPS = (1024, 960, 128)

_compiled = {}
LAST_RESULT = None  # BassKernelResults of the most recent run (for test harness)


CW = 256  # moving-dim chunk width


def _chunks_of(cap):
    """Split a slot capacity into <=CW-wide chunks."""
    out, off = [], 0
    while off < cap:
        w = min(CW, cap - off)
        out.append((off, w))
        off += w
    return out


def _build(caps=CAPS):
    if caps in _compiled:
        return _compiled[caps]
    f32 = mybir.dt.float32
    bf = mybir.dt.bfloat16
    f8 = mybir.dt.float8e4
    relu = mybir.ActivationFunctionType.Relu
    C = sum(caps)
    offs = [sum(caps[:i]) for i in range(len(caps))]

    nc = bacc.Bacc(None, target_bir_lowering=False)
    # x transposed per k-tile: xt[p, k, c] = x_tok_c[k*128+p]
    xt_d = nc.dram_tensor("xt", [P, KD, C], bf, kind="ExternalInput")
    # w1[s, p, hm, k, j] = W1[e_s][k*128+p, hm*128+j]
    w1_d = nc.dram_tensor("w1", [3, P, KH, KD, P], bf, kind="ExternalInput")
    b1_d = nc.dram_tensor("b1", [3, P, KH], f32, kind="ExternalInput")
    # w2[s, dn, p, hm, j] = W2[e_s][hm*128+p, dn*128+j]  (slots 0/1, bf16)
    w2_d = nc.dram_tensor("w2", [2, KD, P, KH, P], bf, kind="ExternalInput")
    # slot 2's W2 in fp8 (scaled by W2Q_SCALE on host)
    w2q_d = nc.dram_tensor("w2q", [KD, P, KH, P], f8, kind="ExternalInput")
    # ye[p, dn, c] = ye_tok_c[dn*128+p]
    ye_d = nc.dram_tensor("ye", [P, KD, C], f32, kind="ExternalOutput")

    with tile.TileContext(nc) as tc:
        with (
            tc.tile_pool(name="xp", bufs=1) as xp,
            tc.tile_pool(name="b1p", bufs=1) as b1p,
            tc.tile_pool(name="w1p", bufs=int(os.environ.get("MOE_W1B", "25"))) as w1p,
            tc.tile_pool(name="w2p", bufs=int(os.environ.get("MOE_W2B", "3"))) as w2p,
            tc.tile_pool(name="w2qp", bufs=int(os.environ.get("MOE_W2QB", "5"))) as w2qp,
            tc.tile_pool(name="hp", bufs=1) as hp,
            tc.tile_pool(name="op", bufs=int(os.environ.get("MOE_OUTB", "4"))) as op,
            tc.tile_pool(name="ps1", bufs=int(os.environ.get("MOE_PS1", "4")), space="PSUM") as ps1,
            tc.tile_pool(name="ps2", bufs=int(os.environ.get("MOE_PS2", "4")), space="PSUM") as ps2,
        ):
            # ---- input staging tiles (x per chunk so readers only wait on
            # their own ~1MB load; b1 as per-slot [128, KH] tiles) ----
            xc = {}  # (slot, chunk_idx) -> tile
            for s, cap in enumerate(caps):
                for ci, (off, w) in enumerate(_chunks_of(cap)):
                    xc[(s, ci)] = xp.tile([P, KD, w], bf, name=f"x{s}_{ci}")

            def load_x(s, ci, halves=1, eng=None):
                off, w = _chunks_of(caps[s])[ci]
                for h in range(halves):
                    k0, k1 = h * KD // halves, (h + 1) * KD // halves
                    (eng or nc.sync).dma_start(
                        xc[(s, ci)][:, k0:k1, :],
                        xt_d[:, k0:k1, offs[s] + off : offs[s] + off + w],
                    )

            b1s = [b1p.tile([P, KH], f32, name=f"b1_{s}") for s in range(3)]

            def load_w1(s, hm, eng=None):
                t = w1p.tile([P, KD, P], bf, tag="w1", name=f"w1_{s}_{hm}")
                (eng or nc.sync).dma_start(t[:], w1_d[s, :, hm])
                return t

            # Startup: scalar ring (otherwise idle until G2) carries b1 + the
            # first W1 tiles while the sync ring streams slot0's x chunks, so
            # the first matmuls and first evictions aren't DMA-serialized.
            load_x(0, 0, halves=2)
            w1_pre = {(0, 0): load_w1(0, 0, eng=nc.scalar)}
            for s in range(3):
                nc.scalar.dma_start(b1s[s][:], b1_d[s])
            load_x(0, 1)
            w1_pre[(0, 1)] = load_w1(0, 1, eng=nc.scalar)
            load_x(0, 2)
            w1_pre[(0, 2)] = load_w1(0, 2, eng=nc.scalar)
            load_x(0, 3)
            w1_pre[(0, 3)] = load_w1(0, 3, eng=nc.scalar)

            def g1(s, hts, extra_dma=None):
                """G1 for slot s: one hm sweep over all its chunks."""
                chunks = _chunks_of(caps[s])
                for hm in range(KH):
                    w1t = w1_pre.pop((s, hm), None) or load_w1(s, hm)
                    pts = []
                    for ci, (off, w) in enumerate(chunks):
                        pt = ps1.tile([P, CW], f32, tag="ps1", name=f"p1_{s}_{hm}_{ci}")
                        for k in range(KD):
                            nc.tensor.matmul(
                                pt[:, :w],
                                w1t[:, k, :],
                                xc[(s, ci)][:, k, :],
                                start=(k == 0),
                                stop=(k == KD - 1),
                            )
                        pts.append(pt)
                    for ci, (off, w) in enumerate(chunks):
                        nc.scalar.activation(
                            hts[ci][:, hm, :w], pts[ci][:, :w], relu,
                            bias=b1s[s][:, hm : hm + 1],
                        )
                    if extra_dma is not None and hm in extra_dma:
                        extra_dma[hm]()

            def g2(s, hts, extra_dma=None):
                """G2 for slot s: dn sweep, W2 streamed (bf16; fp8 for slot2)."""
                chunks = _chunks_of(caps[s])
                fp8 = s == 2
                w2ts = {}

                def load_w2(dn):
                    if fp8:
                        t = w2qp.tile([P, KH, P], f8, tag="w2q", name=f"w2q{dn}")
                        nc.sync.dma_start(t[:], w2q_d[dn])
                        return t
                    t = w2p.tile([P, KH, P], bf, tag="w2", name=f"w2_{s}_{dn}")
                    nc.sync.dma_start(t[:], w2_d[s, dn])
                    return t

                npre = 3
                for dn in range(min(npre, KD)):
                    w2ts[dn] = load_w2(dn)
                for dn in range(KD):
                    w2t = w2ts.pop(dn) if dn in w2ts else load_w2(dn)
                    if dn + npre < KD:
                        w2ts[dn + npre] = load_w2(dn + npre)
                    if extra_dma is not None and dn in extra_dma:
                        extra_dma[dn]()
                    for ci, (off, w) in enumerate(chunks):
                        pt = ps2.tile([P, CW], f32, tag="ps2", name=f"p2_{s}_{dn}_{ci}")
                        for hm in range(KH):
                            nc.tensor.matmul(
                                pt[:, :w],
                                w2t[:, hm, :],
                                hts[ci][:, hm, :w],
                                start=(hm == 0),
                                stop=(hm == KH - 1),
                            )
                        ot = op.tile([P, CW], f32, tag="ot", name=f"o_{s}_{dn}_{ci}")
                        nc.vector.tensor_copy(ot[:, :w], pt[:, :w])
                        nc.gpsimd.dma_start(
                            ye_d[:, dn, offs[s] + off : offs[s] + off + w], ot[:, :w]
                        )

            # hid tiles: fp8 for slot2; bf16 tags reused between slots 0/1
            hq = [hp.tile([P, KH, caps[2]], f8, name="hq")]
            hts01 = [hp.tile([P, KH, CW], bf, name=f"h{i}") for i in range(4)]

            # x prefetch interleave: stage later slots' x chunks inside the
            # first G1 sweeps so they don't delay the W1 streams.
            g1(0, hts01, extra_dma={
                8: lambda: load_x(1, 0),
                14: lambda: load_x(1, 1),
                20: lambda: load_x(1, 2),
                26: lambda: load_x(1, 3),
            })
            g2(0, hts01)
            g1(1, hts01, extra_dma={10: lambda: load_x(2, 0)})
            g2(1, hts01)
            g1(2, hq)
            g2(2, hq)

    nc.compile()
    _compiled[caps] = nc
    return nc


def _route(xf, Wg, bg):
    """fp64 gating: returns top-2 expert ids + gate weights per token."""
    logits = xf.astype(np.float64) @ Wg.astype(np.float64) + bg.astype(np.float64)
    mx = logits.max(axis=1, keepdims=True)
    ex = np.exp(logits - mx)
    probs = ex / ex.sum(axis=1, keepdims=True)
    order = np.argsort(-logits, axis=1, kind="stable")  # ties -> lower index
    top = order[:, :TOPK]
    gsel = np.take_along_axis(probs, top, axis=1).astype(np.float32)
    return top, gsel


def _pack(counts, caps):
    """Assign experts to the 8*len(caps) slot groups.

    Returns groups[core][slot] = expert id (or -1 unused), and per-expert list
    of (core, slot, n_tokens) in fill order.  None if infeasible.
    """
    ncaps = len(caps)
    owner = [[-1] * ncaps for _ in range(NCORES)]
    parts = [[] for _ in range(E)]
    rem = list(counts)
    # Phase 1: own core's slots, big to small, only as needed.
    for e in range(E):
        for s in range(ncaps):
            if rem[e] <= 0:
                break
            take = min(rem[e], caps[s])
            owner[e][s] = e
            parts[e].append([e, s, take])
            rem[e] -= take
    # Phase 2: spill into unused slots elsewhere (smallest sufficient first).
    free = [
        (c, s) for c in range(NCORES) for s in range(ncaps) if owner[c][s] < 0
    ]
    free.sort(key=lambda cs: caps[cs[1]])
    for e in sorted(range(E), key=lambda e: -rem[e]):
        while rem[e] > 0:
            pick = None
            for i, (c, s) in enumerate(free):
                if caps[s] >= rem[e]:
                    pick = i
                    break
            if pick is None:
                pick = len(free) - 1 if free else None
            if pick is None:
                return None, None
            c, s = free.pop(pick)
            take = min(rem[e], caps[s])
            owner[c][s] = e
            parts[e].append([c, s, take])
            rem[e] -= take
    return owner, parts


def kernel(x, Wg, bg, W1, b1, W2, b2):
    global LAST_RESULT
    x = np.ascontiguousarray(x, dtype=np.float32)
    B, S, d = x.shape
    assert d == D
    T = B * S
    xf = x.reshape(T, d)

    top, gsel = _route(xf, Wg, bg)
    toks, gates = [], []
    for e in range(E):
        pos = top == e
        sel = pos.any(axis=1)
        toks.append(np.nonzero(sel)[0])
        gates.append((gsel * pos).sum(axis=1)[sel].astype(np.float32))

    counts = [len(t) for t in toks]
    caps = CAPS
    owner = parts = None
    # Escalate capacities if this routing distribution doesn't pack.
    for caps_try in [CAPS, (1280, 1152, 256), (2432, 2176, 256)]:
        owner, parts = _pack(counts, caps_try)
        if owner is not None:
            caps = caps_try
            break
    assert owner is not None, "packing failed"
    offs = [sum(caps[:i]) for i in range(len(caps))]
    C = sum(caps)

    nc = _build(caps)

    # Host-side weight prep (per expert, reused across slots)
    w1_prep, w2_prep, w2q_prep, b1_prep = {}, {}, {}, {}
    for e in set(ow for row in owner for ow in row if ow >= 0):
        W1e = np.asarray(W1[e], dtype=np.float32)
        W2e = np.asarray(W2[e], dtype=np.float32)
        w1_prep[e] = np.ascontiguousarray(
            W1e.reshape(KD, P, KH, P).transpose(1, 2, 0, 3)
        ).astype(BF16)
        w2_prep[e] = np.ascontiguousarray(
            W2e.reshape(KH, P, KD, P).transpose(2, 1, 0, 3)
        ).astype(BF16)
        w2q_prep[e] = (
            np.ascontiguousarray(W2e.reshape(KH, P, KD, P).transpose(2, 1, 0, 3))
            * W2Q_SCALE
        ).astype(FP8)
        b1_prep[e] = np.ascontiguousarray(
            np.asarray(b1[e], dtype=np.float32).reshape(KH, P).T
        )

    # Token fill: walk parts in order, slicing each expert's token list.
    fill = [[None] * len(caps) for _ in range(NCORES)]  # (expert, tok_idx_array)
    for e in range(E):
        off = 0
        for c, s, n in parts[e]:
            fill[c][s] = (e, toks[e][off : off + n])
            off += n

    in_maps = []
    for c in range(NCORES):
        xt = np.zeros((P, KD, C), BF16)
        w1m = np.zeros((3, P, KH, KD, P), BF16)
        b1m = np.zeros((3, P, KH), np.float32)
        w2m = np.zeros((2, KD, P, KH, P), BF16)
        w2qm = np.zeros((KD, P, KH, P), FP8)
        for s in range(len(caps)):
            ent = fill[c][s]
            if ent is None:
                continue
            e, tk = ent
            xe = np.zeros((caps[s], D), np.float32)
            xe[: len(tk)] = xf[tk]
            xt[:, :, offs[s] : offs[s] + caps[s]] = (
                xe.T.reshape(KD, P, caps[s]).transpose(1, 0, 2).astype(BF16)
            )
            w1m[s] = w1_prep[e]
            b1m[s] = b1_prep[e]
            if s < 2:
                w2m[s] = w2_prep[e]
            else:
                w2qm[:] = w2q_prep[e]
        in_maps.append({"xt": xt, "w1": w1m, "b1": b1m, "w2": w2m, "w2q": w2qm})

    res = run_bass_kernel_spmd(nc, in_maps, core_ids=list(range(NCORES)))
    LAST_RESULT = res

    out = np.zeros((T, D), np.float32)
    b2f = np.asarray(b2, dtype=np.float32)
    gate_off = [0] * E
    for e in range(E):
        off = 0
        for c, s, n in parts[e]:
            if n == 0:
                continue
            ye = res.results[c]["ye"][:, :, offs[s] : offs[s] + n]  # [P, KD, n]
            ye = ye.transpose(2, 1, 0).reshape(n, D).astype(np.float32)
            if s == 2:
                ye = ye / W2Q_SCALE
            tk = toks[e][off : off + n]
            g = gates[e][off : off + n]
            out[tk] += g[:, None] * (ye + b2f[e])
            off += n
    return out.reshape(B, S, D)


# revision 29
# speedup vs baseline: 1.1766x; 1.0147x over previous
"""MoE (top-2 of 8 experts, d=1024, h=4096) on 8 Trainium2 NeuronCores.

Strategy (expert-parallel with load-balanced spill slots):
  - Host: gating (fp64 logits/softmax/top-2; tie margins ~1e-5 >> fp32 noise).
  - Each core runs the same program with three single-expert "slots" of fixed
    token capacity (1024, 960, 128) = C 2112.  Each expert fills its own
    core's slots first; the three oversized experts spill their overflow into
    128-token slots borrowed from underloaded cores.  This caps the per-core
    capacity near the mean (2112 vs 2304 for naive expert-per-core).
  - Device per slot (fused, hid stays in SBUF): G1: hidT = relu(W1_s.T @ x_s
    + b1_s) per 128-row tile straight out of PSUM; G2: ye = hidT.T @ W2_s with
    W2 streamed.  All matmuls bf16 (slot 2's G2 in fp8e4 so its weights can be
    fully prefetched; only ~450 of 16384 token-assignments touch fp8).
  - Host: out[tok] += gate * (ye + b2)  (scatter-combine, fp32).

Self-contained: hardcodes all shapes; only imports concourse (system lib).
"""

import os

os.environ.setdefault("JAX_PLATFORMS", "")

import numpy as np
import ml_dtypes

import concourse.bacc as bacc
import concourse.mybir as mybir
import concourse.tile as tile
from concourse.bass_utils import run_bass_kernel_spmd

P = 128
D = 1024  # embed dim
H = 4096  # hidden dim
E = 8  # experts
TOPK = 2
KD = D // P  # 8  k-tiles over embed
KH = H // P  # 32 k-tiles over hidden
NCORES = 8

BF16 = ml_dtypes.bfloat16
FP8 = ml_dtypes.float8_e4m3
W2Q_SCALE = 64.0  # host pre-scale so fp8 W2 values sit in the normal range
W1Q_SCALE = 32.0  # same for slot2's fp8 W1

# Per-core slot capacities.  Slot 2 (the spill slot) runs its G2 in fp8.
CAPS = (1024, 960, 128)

_compiled = {}
LAST_RESULT = None  # BassKernelResults of the most recent run (for test harness)


CW = 256  # moving-dim chunk width


def _chunks_of(cap):
    """Split a slot capacity into <=CW-wide chunks."""
    out, off = [], 0
    while off < cap:
        w = min(CW, cap - off)
        out.append((off, w))
        off += w
    return out


def _build(caps=CAPS):
    if caps in _compiled:
        return _compiled[caps]
    f32 = mybir.dt.float32
    bf = mybir.dt.bfloat16
    f8 = mybir.dt.float8e4
    relu = mybir.ActivationFunctionType.Relu
    C = sum(caps)
    offs = [sum(caps[:i]) for i in range(len(caps))]

    nc = bacc.Bacc(None, target_bir_lowering=False)
    # x transposed per k-tile: xt[p, k, c] = x_tok_c[k*128+p]
    xt_d = nc.dram_tensor("xt", [P, KD, C], bf, kind="ExternalInput")
    # w1[s, p, hm, k, j] = W1[e_s][k*128+p, hm*128+j]
    w1_d = nc.dram_tensor("w1", [3, P, KH, KD, P], bf, kind="ExternalInput")
    b1_d = nc.dram_tensor("b1", [3, P, KH], f32, kind="ExternalInput")
    # w2[s, dn, p, hm, j] = W2[e_s][hm*128+p, dn*128+j]  (slots 0/1, bf16)
    w2_d = nc.dram_tensor("w2", [2, KD, P, KH, P], bf, kind="ExternalInput")
    # slot 2's W2 in fp8 (scaled by W2Q_SCALE on host)
    w2q_d = nc.dram_tensor("w2q", [KD, P, KH, P], f8, kind="ExternalInput")
    # slot 2's W1 (scaled by W1Q_SCALE) and x in fp8 for DoubleRow G1
    w1q_d = nc.dram_tensor("w1q", [P, KH, KD, P], f8, kind="ExternalInput")
    xq_d = nc.dram_tensor("xq", [P, KD, caps[2]], f8, kind="ExternalInput")
    # ye[p, dn, c] = ye_tok_c[dn*128+p]
    ye_d = nc.dram_tensor("ye", [P, KD, C], f32, kind="ExternalOutput")

    with tile.TileContext(nc) as tc:
        with (
            tc.tile_pool(name="xp", bufs=1) as xp,
            tc.tile_pool(name="b1p", bufs=1) as b1p,
            tc.tile_pool(name="w1p", bufs=int(os.environ.get("MOE_W1B", "15"))) as w1p,
            tc.tile_pool(name="w1qp", bufs=KH) as w1qp,
            tc.tile_pool(name="w2p", bufs=int(os.environ.get("MOE_W2B", "3"))) as w2p,
            tc.tile_pool(name="w2qp", bufs=int(os.environ.get("MOE_W2QB", "3"))) as w2qp,
            tc.tile_pool(name="hp", bufs=1) as hp,
            tc.tile_pool(name="op", bufs=int(os.environ.get("MOE_OUTB", "4"))) as op,
            tc.tile_pool(name="ps1", bufs=int(os.environ.get("MOE_PS1", "4")), space="PSUM") as ps1,
            tc.tile_pool(name="ps2", bufs=int(os.environ.get("MOE_PS2", "4")), space="PSUM") as ps2,
        ):
            # ---- input staging tiles (x per chunk so readers only wait on
            # their own ~1MB load; b1 as per-slot [128, KH] tiles) ----
            xc = {}  # (slot, chunk_idx) -> tile
            for s, cap in enumerate(caps):
                if s == 2:
                    continue
                for ci, (off, w) in enumerate(_chunks_of(cap)):
                    xc[(s, ci)] = xp.tile([P, KD, w], bf, name=f"x{s}_{ci}")

            def load_x(s, ci, halves=1, eng=None):
                off, w = _chunks_of(caps[s])[ci]
                for h in range(halves):
                    k0, k1 = h * KD // halves, (h + 1) * KD // halves
                    (eng or nc.sync).dma_start(
                        xc[(s, ci)][:, k0:k1, :],
                        xt_d[:, k0:k1, offs[s] + off : offs[s] + off + w],
                    )

            b1s = [b1p.tile([P, KH], f32, name=f"b1_{s}") for s in range(3)]
            xq = xp.tile([P, KD, caps[2]], f8, name="xq")

            def load_xq():
                nc.sync.dma_start(xq[:], xq_d[:])

            def load_w1(s, hm, eng=None):
                t = w1p.tile([P, KD, P], bf, tag="w1", name=f"w1_{s}_{hm}")
                (eng or nc.sync).dma_start(t[:], w1_d[s, :, hm])
                return t

            # Startup: scalar ring (otherwise idle until G2) carries b1 + the
            # first W1 tiles while the sync ring streams slot0's x chunks, so
            # the first matmuls and first evictions aren't DMA-serialized.
            load_x(0, 0, halves=2)
            w1_pre = {(0, 0): load_w1(0, 0, eng=nc.scalar)}
            for s in range(3):
                nc.scalar.dma_start(b1s[s][:], b1_d[s])
            for ci in range(1, len(_chunks_of(caps[0]))):
                load_x(0, ci)
                if ci < 4:
                    w1_pre[(0, ci)] = load_w1(0, ci, eng=nc.scalar)

            def g1(s, hts, extra_dma=None):
                """G1 for slot s: one hm sweep over all its chunks."""
                chunks = _chunks_of(caps[s])
                fp8 = s == 2
                for hm in range(KH):
                    if fp8:
                        w1t = w1qp.tile([P, KD, P], f8, tag="w1q", name=f"w1q_{hm}")
                        nc.sync.dma_start(w1t[:], w1q_d[:, hm])
                    else:
                        w1t = w1_pre.pop((s, hm), None) or load_w1(s, hm)
                    pts = []
                    for ci, (off, w) in enumerate(chunks):
                        pt = ps1.tile([P, CW], f32, tag="ps1", name=f"p1_{s}_{hm}_{ci}")
                        if fp8:
                            for k2 in range(KD // 2):
                                nc.tensor.matmul(
                                    pt[:, :w],
                                    w1t[:, 2 * k2 : 2 * k2 + 2, :],
                                    xq[:, 2 * k2 : 2 * k2 + 2, :],
                                    start=(k2 == 0),
                                    stop=(k2 == KD // 2 - 1),
                                    perf_mode=mybir.MatmulPerfMode.DoubleRow,
                                )
                        else:
                            for k in range(KD):
                                nc.tensor.matmul(
                                    pt[:, :w],
                                    w1t[:, k, :],
                                    xc[(s, ci)][:, k, :],
                                    start=(k == 0),
                                    stop=(k == KD - 1),
                                )
                        pts.append(pt)
                    for ci, (off, w) in enumerate(chunks):
                        nc.scalar.activation(
                            hts[ci][:, hm, :w], pts[ci][:, :w], relu,
                            bias=b1s[s][:, hm : hm + 1],
                            scale=(1.0 / W1Q_SCALE) if fp8 else 1.0,
                        )
                    if extra_dma is not None and hm in extra_dma:
                        extra_dma[hm]()

            def g2(s, hts, extra_dma=None, split_last=1):
                """G2 for slot s: dn sweep, W2 streamed (bf16; fp8 for slot2).

                split_last subdivides the final chunk so the tail eviction/DMA
                chain overlaps the last matmuls.
                """
                chunks = [[ci, 0, off, w] for ci, (off, w) in enumerate(_chunks_of(caps[s]))]
                if split_last > 1:
                    ci, _, off, w = chunks.pop()
                    h = w // split_last
                    for j in range(split_last):
                        chunks.append([ci, j * h, off + j * h, w - (split_last - 1) * h if j == split_last - 1 else h])
                fp8 = s == 2
                w2ts = {}

                def load_w2(dn):
                    if fp8:
                        t = w2qp.tile([P, KH, P], f8, tag="w2q", name=f"w2q{dn}")
                        nc.sync.dma_start(t[:], w2q_d[dn])
                        return t
                    t = w2p.tile([P, KH, P], bf, tag="w2", name=f"w2_{s}_{dn}")
                    nc.sync.dma_start(t[:], w2_d[s, dn])
                    return t

                npre = 3
                for dn in range(min(npre, KD)):
                    w2ts[dn] = load_w2(dn)
                for dn in range(KD):
                    w2t = w2ts.pop(dn) if dn in w2ts else load_w2(dn)
                    if dn + npre < KD:
                        w2ts[dn + npre] = load_w2(dn + npre)
                    if extra_dma is not None and dn in extra_dma:
                        extra_dma[dn]()
                    for j, (ci, toff, off, w) in enumerate(chunks):
                        pt = ps2.tile([P, CW], f32, tag="ps2", name=f"p2_{s}_{dn}_{j}")
                        if fp8:
                            # fp8 DoubleRow: two packed h-tiles per matmul
                            for h2 in range(KH // 2):
                                nc.tensor.matmul(
                                    pt[:, :w],
                                    w2t[:, 2 * h2 : 2 * h2 + 2, :],
                                    hts[ci][:, 2 * h2 : 2 * h2 + 2, toff : toff + w],
                                    start=(h2 == 0),
                                    stop=(h2 == KH // 2 - 1),
                                    perf_mode=mybir.MatmulPerfMode.DoubleRow,
                                )
                        else:
                            for hm in range(KH):
                                nc.tensor.matmul(
                                    pt[:, :w],
                                    w2t[:, hm, :],
                                    hts[ci][:, hm, toff : toff + w],
                                    start=(hm == 0),
                                    stop=(hm == KH - 1),
                                )
                        ot = op.tile([P, CW], f32, tag="ot", name=f"o_{s}_{dn}_{j}")
                        nc.vector.tensor_copy(ot[:, :w], pt[:, :w])
                        out_eng = nc.scalar if (fp8 and dn == KD - 1) else nc.gpsimd
                        out_eng.dma_start(
                            ye_d[:, dn, offs[s] + off : offs[s] + off + w], ot[:, :w]
                        )

            # hid tiles: fp8 for slot2; bf16 tags reused between slots 0/1
            hq = [hp.tile([P, KH, caps[2]], f8, name="hq")]
            n_hts = max(len(_chunks_of(caps[0])), len(_chunks_of(caps[1])))
            hts01 = [hp.tile([P, KH, CW], bf, name=f"h{i}") for i in range(n_hts)]

            # x prefetch interleave: stage later slots' x chunks inside the
            # first G1 sweeps so they don't delay the W1 streams.
            n1 = len(_chunks_of(caps[1]))
            g1(0, hts01, extra_dma={
                8 + 6 * i: (lambda ci: lambda: load_x(1, ci))(i) for i in range(n1)
            })
            g2(0, hts01)
            g1(1, hts01, extra_dma={10: load_xq})
            g2(1, hts01)
            g1(2, hq)
            g2(2, hq)

    nc.compile()
    _compiled[caps] = nc
    return nc


def _route(xf, Wg, bg):
    """fp64 gating: returns top-2 expert ids + gate weights per token."""
    logits = xf.astype(np.float64) @ Wg.astype(np.float64) + bg.astype(np.float64)
    mx = logits.max(axis=1, keepdims=True)
    ex = np.exp(logits - mx)
    probs = ex / ex.sum(axis=1, keepdims=True)
    order = np.argsort(-logits, axis=1, kind="stable")  # ties -> lower index
    top = order[:, :TOPK]
    gsel = np.take_along_axis(probs, top, axis=1).astype(np.float32)
    return top, gsel


def _pack(counts, caps):
    """Assign experts to the 8*len(caps) slot groups.

    Returns groups[core][slot] = expert id (or -1 unused), and per-expert list
    of (core, slot, n_tokens) in fill order.  None if infeasible.
    """
    ncaps = len(caps)
    owner = [[-1] * ncaps for _ in range(NCORES)]
    parts = [[] for _ in range(E)]
    rem = list(counts)
    # Phase 1: own core's slots, big to small, only as needed.
    for e in range(E):
        for s in range(ncaps):
            if rem[e] <= 0:
                break
            take = min(rem[e], caps[s])
            owner[e][s] = e
            parts[e].append([e, s, take])
            rem[e] -= take
    # Phase 2: spill into unused slots elsewhere (smallest sufficient first).
    free = [
        (c, s) for c in range(NCORES) for s in range(ncaps) if owner[c][s] < 0
    ]
    free.sort(key=lambda cs: caps[cs[1]])
    for e in sorted(range(E), key=lambda e: -rem[e]):
        while rem[e] > 0:
            pick = None
            for i, (c, s) in enumerate(free):
                if caps[s] >= rem[e]:
                    pick = i
                    break
            if pick is None:
                pick = len(free) - 1 if free else None
            if pick is None:
                return None, None
            c, s = free.pop(pick)
            take = min(rem[e], caps[s])
            owner[c][s] = e
            parts[e].append([c, s, take])
            rem[e] -= take
    return owner, parts


def kernel(x, Wg, bg, W1, b1, W2, b2):
    global LAST_RESULT
    x = np.ascontiguousarray(x, dtype=np.float32)
    B, S, d = x.shape
    assert d == D
    T = B * S
    xf = x.reshape(T, d)

    top, gsel = _route(xf, Wg, bg)
    toks, gates = [], []
    for e in range(E):
        pos = top == e
        sel = pos.any(axis=1)
        toks.append(np.nonzero(sel)[0])
        gates.append((gsel * pos).sum(axis=1)[sel].astype(np.float32))

    counts = [len(t) for t in toks]
    caps = CAPS
    owner = parts = None
    # Escalate capacities if this routing distribution doesn't pack.
    for caps_try in [CAPS, (1280, 1152, 256), (2432, 2176, 256)]:
        owner, parts = _pack(counts, caps_try)
        if owner is not None:
            caps = caps_try
            break
    assert owner is not None, "packing failed"
    offs = [sum(caps[:i]) for i in range(len(caps))]
    C = sum(caps)

    nc = _build(caps)

    # Host-side weight prep (per expert, reused across slots)
    w1_prep, w2_prep, w2q_prep, w1q_prep, b1_prep = {}, {}, {}, {}, {}
    for e in set(ow for row in owner for ow in row if ow >= 0):
        W1e = np.asarray(W1[e], dtype=np.float32)
        W2e = np.asarray(W2[e], dtype=np.float32)
        w1_prep[e] = np.ascontiguousarray(
            W1e.reshape(KD, P, KH, P).transpose(1, 2, 0, 3)
        ).astype(BF16)
        w2_prep[e] = np.ascontiguousarray(
            W2e.reshape(KH, P, KD, P).transpose(2, 1, 0, 3)
        ).astype(BF16)
        w1q_prep[e] = (
            np.ascontiguousarray(W1e.reshape(KD, P, KH, P).transpose(1, 2, 0, 3))
            * W1Q_SCALE
        ).astype(FP8)
        w2q_prep[e] = (
            np.ascontiguousarray(W2e.reshape(KH, P, KD, P).transpose(2, 1, 0, 3))
            * W2Q_SCALE
        ).astype(FP8)
        b1_prep[e] = np.ascontiguousarray(
            np.asarray(b1[e], dtype=np.float32).reshape(KH, P).T
        )

    # Token fill: walk parts in order, slicing each expert's token list.
    fill = [[None] * len(caps) for _ in range(NCORES)]  # (expert, tok_idx_array)
    for e in range(E):
        off = 0
        for c, s, n in parts[e]:
            fill[c][s] = (e, toks[e][off : off + n])
            off += n

    in_maps = []
    for c in range(NCORES):
        xt = np.zeros((P, KD, C), BF16)
        w1m = np.zeros((3, P, KH, KD, P), BF16)
        b1m = np.zeros((3, P, KH), np.float32)
        w2m = np.zeros((2, KD, P, KH, P), BF16)
        w2qm = np.zeros((KD, P, KH, P), FP8)
        w1qm = np.zeros((P, KH, KD, P), FP8)
        xqm = np.zeros((P, KD, caps[2]), FP8)
        for s in range(len(caps)):
            ent = fill[c][s]
            if ent is None:
                continue
            e, tk = ent
            xe = np.zeros((caps[s], D), np.float32)
            xe[: len(tk)] = xf[tk]
            xt[:, :, offs[s] : offs[s] + caps[s]] = (
                xe.T.reshape(KD, P, caps[s]).transpose(1, 0, 2).astype(BF16)
            )
            w1m[s] = w1_prep[e]
            b1m[s] = b1_prep[e]
            if s < 2:
                w2m[s] = w2_prep[e]
            else:
                w2qm[:] = w2q_prep[e]
                w1qm[:] = w1q_prep[e]
                xqm[:] = (
                    xe.T.reshape(KD, P, caps[s]).transpose(1, 0, 2).astype(FP8)
                )
        in_maps.append(
            {"xt": xt, "w1": w1m, "b1": b1m, "w2": w2m, "w2q": w2qm,
             "w1q": w1qm, "xq": xqm}
        )

    res = run_bass_kernel_spmd(nc, in_maps, core_ids=list(range(NCORES)))
    LAST_RESULT = res

    out = np.zeros((T, D), np.float32)
    b2f = np.asarray(b2, dtype=np.float32)
    gate_off = [0] * E
    for e in range(E):
        off = 0
        for c, s, n in parts[e]:
            if n == 0:
                continue
            ye = res.results[c]["ye"][:, :, offs[s] : offs[s] + n]  # [P, KD, n]
            ye = ye.transpose(2, 1, 0).reshape(n, D).astype(np.float32)
            if s == 2:
                ye = ye / W2Q_SCALE
            tk = toks[e][off : off + n]
            g = gates[e][off : off + n]
            out[tk] += g[:, None] * (ye + b2f[e])
            off += n
    return out.reshape(B, S, D)
